# revision 2
# baseline (speedup 1.0000x reference)
"""Trainium2 Bass kernel for a binary (1w1a) depthwise-separable conv block.

Reference computation (NCHW, B=32, C=CO=512, H=W=56):
    xb  = sign(x)
    y1  = depthwise_conv3x3(xb, sign(w_dw), pad=1)          # per-channel
    z   = sign(y1 * s1 + t1)                                # BN1 + binarize
    y2  = pointwise_conv1x1(z, sign(w_pw))                  # dense 512->512
    out = y2 * s2 + t2                                      # BN2

Sharding: data-parallel over batch, 4 images per core on 8 cores.

Key implementation points:
  - Inputs arrive as bf16 (sign-exact), outputs leave as bf16 and are upcast
    to fp32 on the host: halves DMA traffic both ways.  bf16 output rounding
    is ~0.2% per element, far inside the 2e-2 gate; everything upstream of
    BN2 is exact integer arithmetic.
  - Depthwise conv on TensorE fp8 DoubleRow: 5 accumulating passes per 8-row
    chunk, each contracting 2 taps.  The two DoubleRow "slots" of the moving
    operand are expressed as OVERLAPPING strided access patterns over a
    single zero-padded sign buffer [128, 60, 60] (slot stride 60 = +1 row,
    2 = +2 cols, 0 = duplicate against a zero stationary slot), so no
    shifted copies are materialized.
  - Sign stages are engine-fungible: ScalarE Sign LUT emits +-1; VectorE
    tensor_scalar (psum >_theta) - 0.5 emits +-0.5 (one instruction), with
    the 2x absorbed per-channel into BN1 scale / doubled (+-2) pointwise
    weights.  BN2 evictions run on either engine (ACT Identity with AP
    scale/bias, or DVE mult+add).  The static assignment below balances
    ScalarE and VectorE at ~80us each, under the ~150us TensorE floor.
  - Pointwise conv: fp8 DoubleRow, 2 passes contracting all 512 channels,
    2-chunk PSUM groups (2 banks) so each BN2 eviction covers 896 elements.
"""

import sys

sys.path.insert(0, "/opt/trn_rl_repo")

from contextlib import ExitStack

import ml_dtypes
import numpy as np

import concourse.bass as bass
import concourse.tile as tile
from concourse import mybir
from concourse.ap import AP
from concourse.bass_utils import run_bass_kernel_spmd

N_CORES = 8
B, C, H, W = 32, 512, 56, 56
CO = 512
EPS = 1e-5
BS = B // N_CORES          # images per core
CG = C // 128              # channel groups
ROWS = 8                   # output rows per PSUM chunk (8*56=448 fp32 <= 1 bank)
NCHUNK = H // ROWS         # 7
PH, PW_ = 60, 60           # padded sign-buffer pitch

F32 = mybir.dt.float32
BF16 = mybir.dt.bfloat16
FP8 = mybir.dt.float8e4
DR = mybir.MatmulPerfMode.DoubleRow
NP_FP8 = ml_dtypes.float8_e4m3
NP_BF16 = ml_dtypes.bfloat16

# Engine assignment knobs (must match between host prep and device build).
SIGN_DVE_CGS = frozenset()          # input-sign on DVE (+-0.5) for these cgs
BN1_DVE_CGS = frozenset({1, 2, 3})  # BN1+sign evict on DVE (+-0.5 z)
BN2_ACT_MOD, BN2_ACT_K = 2, 1       # BN2 evict on ACT when ctr % MOD < K


def _legalize_sem_waits(nc, max_waits=1):
    """walrus (CoreV3 codegen) rejects instructions carrying more than one
    sync-wait command.  Tile's kernel-tail drain waits on every outstanding
    semaphore at once; split excess waits onto preceding no-ops on the same
    engine (engines execute their stream in order, so blocking semantics are
    identical)."""
    n_split = 0
    for f in nc.m.functions:
        for bb in f.blocks:
            insts = bb.instructions
            newlist = []
            for inst in insts:
                si = inst.sync_info
                waits = list(si.on_wait) if si is not None else []
                if len(waits) > max_waits:
                    excess, keep = waits[:-max_waits], waits[-max_waits:]
                    for k, w in enumerate(excess):
                        sp = mybir.InstNoOp(name=f"{inst.name}-lgw{k}")
                        sp.engine = inst.engine
                        sp.sync_info = mybir.SyncInfo(on_wait=[w], on_update=[])
                        newlist.append(sp)
                        n_split += 1
                    inst.sync_info = mybir.SyncInfo(
                        on_wait=keep, on_update=list(si.on_update)
                    )
                newlist.append(inst)
            insts[:] = newlist
    return n_split


def build_bass():
    nc = bass.Bass("TRN2", target_bir_lowering=False, debug=False)

    x_d = nc.dram_tensor("x", [BS, C, H, W], BF16, kind="ExternalInput")
    # dw pairs: idx = cg*5 + p; p in 0..2 -> taps (0,p)&(1,p) [slot stride 60];
    # p=3 -> taps (2,0)&(2,2) [slot stride 2]; p=4 -> tap (2,1)&zero [stride 0]
    wdw_d = nc.dram_tensor("wdw", [128, CG * 5, 2, 128], FP8, kind="ExternalInput")
    # pw pairs: idx = zpair*CG + cob; slot j of zpair holds channels
    # (zpair*2+j)*128 ..
    wpw_d = nc.dram_tensor("wpw", [128, 2 * CG, 2, 128], FP8, kind="ExternalInput")
    bn1_d = nc.dram_tensor("bn1", [128, 2 * CG], F32, kind="ExternalInput")
    bn2_d = nc.dram_tensor("bn2", [128, 2 * CG], F32, kind="ExternalInput")
    y_d = nc.dram_tensor("y", [BS, CO, H, W], BF16, kind="ExternalOutput")

    SIGN = mybir.ActivationFunctionType.Sign
    IDENT = mybir.ActivationFunctionType.Identity
    MULT = mybir.AluOpType.mult
    ADD = mybir.AluOpType.add
    GT = mybir.AluOpType.is_gt
    SUB = mybir.AluOpType.subtract

    with tile.TileContext(nc) as tc:
        with ExitStack() as ctx:
            const = ctx.enter_context(tc.tile_pool(name="const", bufs=1))
            xin_pool = ctx.enter_context(tc.tile_pool(name="xin", bufs=8))

            # Prefetch the whole first image before the (bulkier) weight DMAs
            # so the Scalar/Vector/PE pipeline can start ASAP.
            xin_tiles = {}
            t = xin_pool.tile([128, H, W], BF16, tag="xin")
            nc.sync.dma_start(t[:, 0:28, :], x_d.ap()[0, 0:128][:, 0:28, :])
            nc.sync.dma_start(t[:, 28:H, :], x_d.ap()[0, 0:128][:, 28:H, :])
            xin_tiles[(0, 0)] = t

            wdw_t = const.tile([128, CG * 5, 2, 128], FP8, tag="wdw")
            for wcg in range(CG):
                nc.sync.dma_start(
                    wdw_t[:, wcg * 5 : (wcg + 1) * 5],
                    wdw_d.ap()[:, wcg * 5 : (wcg + 1) * 5],
                )
            for pcg in range(1, CG):
                t = xin_pool.tile([128, H, W], BF16, tag="xin")
                nc.sync.dma_start(t[:], x_d.ap()[0, pcg * 128 : (pcg + 1) * 128])
                xin_tiles[(0, pcg)] = t
            wpw_t = const.tile([128, 2 * CG, 2, 128], FP8, tag="wpw")
            nc.sync.dma_start(wpw_t[:], wpw_d.ap()[:])
            bn1_t = const.tile([128, 2 * CG], F32, tag="bn1")
            nc.sync.dma_start(bn1_t[:], bn1_d.ap()[:])
            bn2_t = const.tile([128, 2 * CG], F32, tag="bn2")
            nc.sync.dma_start(bn2_t[:], bn2_d.ap()[:])

            # persistent padded sign(x) buffers [128, 60, 60] fp8; only the
            # borders (row 0, row 57, col 0, col 57) need one-time zeros --
            # the interior is rewritten by the per-iteration sign op, and
            # rows/cols beyond 57 are never read.
            xpads = []
            for k in range(3):
                xpa = const.tile([128, PH, PW_], FP8, tag=f"xpad{k}")
                flat = xpa[:].rearrange("p a b -> p (a b)")
                # row 0 (cols 0..59) + row 1 col 0
                nc.vector.memset(flat[:, 0:61], 0)
                # row 57
                nc.vector.memset(flat[:, 57 * 60 : 57 * 60 + 60], 0)
                # rows 1..56: cols 57,58,59 + next row's col 0
                bm = AP(flat.tensor, 60 + 57, [list(flat.ap[0]), [60, 56], [1, 4]])
                nc.vector.memset(bm, 0)
                xpads.append(xpa)

            z_pool = ctx.enter_context(tc.tile_pool(name="z", bufs=4))
            out_pool = ctx.enter_context(tc.tile_pool(name="outb", bufs=2))
            psdw_pool = ctx.enter_context(
                tc.tile_pool(name="psdw", bufs=2, space="PSUM")
            )
            pspw_pool = ctx.enter_context(
                tc.tile_pool(name="pspw", bufs=2, space="PSUM")
            )

            prepared = {}

            def emit_sign(dst, src, cg):
                if cg in SIGN_DVE_CGS:
                    nc.vector.tensor_scalar(dst, src, 0.0, 0.5, GT, SUB)
                else:
                    nc.scalar.activation(dst, src, SIGN)

            def prepare(bp, cgp):
                """Emit the sign stage for iteration (bp,cgp), one iteration
                ahead of the matmul consumer so it never head-of-line-blocks
                the z-evictions that gate PSUM reuse."""
                itp = bp * CG + cgp
                xin = xin_tiles.pop((bp, cgp))
                xpa = xpads[itp % 3]
                if itp == 0:
                    emit_sign(xpa[:, 1:29, 1 : W + 1], xin[:, 0:28, :], cgp)
                    emit_sign(xpa[:, 29 : H + 1, 1 : W + 1], xin[:, 28:H, :], cgp)
                else:
                    emit_sign(xpa[:, 1 : H + 1, 1 : W + 1], xin[:], cgp)
                prepared[(bp, cgp)] = xpa

            def mov(xpa, r0, co, ds):
                """DoubleRow moving AP over the padded buffer: slot stride ds,
                8 rows of 56 from padded row r0, col co."""
                base = xpa[:]
                return AP(
                    base.tensor,
                    base.offset + PH * r0 + co,
                    [list(base.ap[0]), [ds, 2], [PW_, 8], [1, 56]],
                )

            zp_hist = {}
            bn2_ctr = [0]

            def emit_pw(bp):
                zpb = zp_hist.pop(bp)
                for cob in range(CG):
                    outb = out_pool.tile([128, H, W], BF16, tag="outb")
                    for g in range(4):
                        members = [2 * g, 2 * g + 1] if g < 3 else [6]
                        m = len(members)
                        pp = pspw_pool.tile([128, 2, 512], F32, tag="pspw")
                        for zpair in range(2):
                            for s, n in enumerate(members):
                                nc.tensor.matmul(
                                    pp[:, s, 0 : ROWS * W],
                                    wpw_t[:, zpair * CG + cob],
                                    zpb[zpair][:, :, n * ROWS : (n + 1) * ROWS, :],
                                    start=(zpair == 0),
                                    stop=(zpair == 1),
                                    perf_mode=DR,
                                )
                        r0 = members[0] * ROWS
                        oout = outb[:, r0 : r0 + m * ROWS, :].rearrange(
                            "p (a r) w -> p a (r w)", a=m
                        )
                        if bn2_ctr[0] % BN2_ACT_MOD < BN2_ACT_K:
                            nc.scalar.activation(
                                oout,
                                pp[:, 0:m, 0 : ROWS * W],
                                IDENT,
                                bias=bn2_t[:, cob * 2 + 1 : cob * 2 + 2],
                                scale=bn2_t[:, cob * 2 : cob * 2 + 1],
                            )
                        else:
                            nc.vector.tensor_scalar(
                                oout,
                                pp[:, 0:m, 0 : ROWS * W],
                                bn2_t[:, cob * 2 : cob * 2 + 1],
                                bn2_t[:, cob * 2 + 1 : cob * 2 + 2],
                                MULT,
                                ADD,
                            )
                        bn2_ctr[0] += 1
                        # stream the output out in halves so the final drain
                        # overlaps compute
                        if g == 1:
                            nc_half = y_d.ap()[bp, cob * 128 : (cob + 1) * 128]
                            nc.sync.dma_start(nc_half[:, 0:32, :], outb[:, 0:32, :])
                    nc.sync.dma_start(
                        y_d.ap()[bp, cob * 128 : (cob + 1) * 128][:, 32:H, :],
                        outb[:, 32:H, :],
                    )

            prepare(0, 0)
            it = 0
            for b in range(BS):
                # prefetch next image's inputs ahead of this image's outputs
                if b + 1 < BS:
                    for pcg in range(CG):
                        t = xin_pool.tile([128, H, W], BF16, tag="xin")
                        nc.sync.dma_start(
                            t[:], x_d.ap()[b + 1, pcg * 128 : (pcg + 1) * 128]
                        )
                        xin_tiles[(b + 1, pcg)] = t
                zp = []
                for _zi in range(2):
                    ztile = z_pool.tile([128, 2, H, W], FP8, tag="z")
                    zp.append(ztile)
                zp_hist[b] = zp
                for cg in range(CG):
                    it += 1
                    # stage the NEXT iteration's sign ahead of this
                    # iteration's matmuls
                    if it < BS * CG:
                        prepare(it // CG, it % CG)
                    xpa = prepared.pop((b, cg))

                    zslot, j = zp[cg // 2], cg % 2
                    # chunk pairs share one 2-bank PSUM tile so the BN1+Sign
                    # eviction reads 2 banks in a single op.  Pass loop is
                    # OUTER so each stationary serves both members
                    # back-to-back.
                    for pg in range(4):
                        members = [2 * pg, 2 * pg + 1] if pg < 3 else [6]
                        m = len(members)
                        ps2 = psdw_pool.tile([128, 2, 512], F32, tag="psdw")
                        # (weight idx, row off, col off, slot stride) per pass
                        passes = [
                            (cg * 5 + 0, 0, 0, PH),
                            (cg * 5 + 1, 0, 1, PH),
                            (cg * 5 + 2, 0, 2, PH),
                            (cg * 5 + 3, 2, 0, 2),
                            (cg * 5 + 4, 2, 1, 0),
                        ]
                        for p, (wi, ro, co, ds) in enumerate(passes):
                            for s, n in enumerate(members):
                                nc.tensor.matmul(
                                    ps2[:, s, 0 : ROWS * W],
                                    wdw_t[:, wi],
                                    mov(xpa, n * ROWS + ro, co, ds),
                                    start=(p == 0),
                                    stop=(p == 4),
                                    perf_mode=DR,
                                )
                        r0 = members[0] * ROWS
                        zout = zslot[:, j, r0 : r0 + ROWS * m, :].rearrange(
                            "p (a r) w -> p a (r w)", a=m
                        )
                        if cg in BN1_DVE_CGS:
                            nc.vector.tensor_scalar(
                                zout,
                                ps2[:, 0:m, 0 : ROWS * W],
                                bn1_t[:, cg * 2 : cg * 2 + 1],
                                0.5,
                                GT,
                                SUB,
                            )
                        else:
                            nc.scalar.activation(
                                zout,
                                ps2[:, 0:m, 0 : ROWS * W],
                                SIGN,
                                bias=bn1_t[:, cg * 2 + 1 : cg * 2 + 2],
                                scale=bn1_t[:, cg * 2 : cg * 2 + 1],
                            )

                    if cg == 0 and b > 0:
                        # previous image's pointwise conv: emitted after
                        # this image's first depthwise block so PE keeps
                        # dense work while the evict engines drain
                        emit_pw(b - 1)

            emit_pw(BS - 1)

    _legalize_sem_waits(nc)
    return nc


_NC_CACHE = None


def _get_nc():
    global _NC_CACHE
    if _NC_CACHE is None:
        _NC_CACHE = build_bass()
    return _NC_CACHE


def make_host_inputs(w_dw, w_pw, g1, b1, m1, v1, g2, b2, m2, v2):
    """Host-side preprocessing shared by all cores (weights/BN constants)."""
    wsign = np.sign(w_dw[:, 0, :, :]).reshape(C, 3, 3).astype(np.float32)

    wdw = np.zeros((128, CG * 5, 2, 128), dtype=NP_FP8)
    idx = np.arange(128)
    for cg in range(CG):
        cs = slice(cg * 128, (cg + 1) * 128)
        for dw in range(3):
            wdw[idx, cg * 5 + dw, 0, idx] = wsign[cs, 0, dw].astype(NP_FP8)
            wdw[idx, cg * 5 + dw, 1, idx] = wsign[cs, 1, dw].astype(NP_FP8)
        # pair 3 (slot stride 2): slot0 = tap (2,0), slot1 = tap (2,2)
        wdw[idx, cg * 5 + 3, 0, idx] = wsign[cs, 2, 0].astype(NP_FP8)
        wdw[idx, cg * 5 + 3, 1, idx] = wsign[cs, 2, 2].astype(NP_FP8)
        # pair 4 (slot stride 0): slot0 = tap (2,1), slot1 stays zero
        wdw[idx, cg * 5 + 4, 0, idx] = wsign[cs, 2, 1].astype(NP_FP8)

    # z scale per input-channel cg: +-0.5 if BN1 evicted on DVE, else +-1;
    # compensated by doubling the pointwise weight for those channels.
    wptT = np.sign(w_pw[:, :, 0, 0]).T.astype(np.float32)  # [c, co]
    for cg in BN1_DVE_CGS:
        wptT[cg * 128 : (cg + 1) * 128, :] *= 2.0
    wpw = np.zeros((128, 2 * CG, 2, 128), dtype=NP_FP8)
    for zpair in range(2):
        for cob in range(CG):
            for j in range(2):
                c0 = (zpair * 2 + j) * 128
                wpw[:, zpair * CG + cob, j, :] = wptT[
                    c0 : c0 + 128, cob * 128 : (cob + 1) * 128
                ].astype(NP_FP8)

    # BN1 constants.  The depthwise psum for cg is alpha*y1 with alpha=0.5
    # when the input sign ran on DVE (+-0.5 xb), else 1.
    s1 = (g1.astype(np.float64) / np.sqrt(v1.astype(np.float64) + EPS)).astype(
        np.float32
    )
    t1 = b1.astype(np.float32) - m1.astype(np.float32) * s1
    bn1 = np.zeros((128, 2 * CG), dtype=np.float32)
    for cg in range(CG):
        cs = slice(cg * 128, (cg + 1) * 128)
        alpha = 0.5 if cg in SIGN_DVE_CGS else 1.0
        if cg in BN1_DVE_CGS:
            # z = (psum > theta') - 0.5 with theta' = -alpha*t1/s1
            s1c = np.maximum(s1[cs], 1e-35)
            theta = -alpha * t1[cs] / s1c
            theta = np.where(
                s1[cs] < 1e-35, np.where(t1[cs] > 0, -1e30, 1e30), theta
            )
            bn1[:, cg * 2] = theta
            bn1[:, cg * 2 + 1] = 0.0
        else:
            bn1[:, cg * 2] = s1[cs] / alpha
            bn1[:, cg * 2 + 1] = t1[cs]

    s2 = (g2.astype(np.float64) / np.sqrt(v2.astype(np.float64) + EPS)).astype(
        np.float32
    )
    t2 = b2.astype(np.float32) - m2.astype(np.float32) * s2
    bn2 = np.zeros((128, 2 * CG), dtype=np.float32)
    for cg in range(CG):
        bn2[:, cg * 2] = s2[cg * 128 : (cg + 1) * 128]
        bn2[:, cg * 2 + 1] = t2[cg * 128 : (cg + 1) * 128]

    return {"wdw": wdw, "wpw": wpw, "bn1": bn1, "bn2": bn2}


def kernel(x, w_dw, w_pw, g1, b1, m1, v1, g2, b2, m2, v2, _trace=False, _tmpdir=None):
    x = np.asarray(x, dtype=np.float32).astype(NP_BF16)
    shared = make_host_inputs(
        np.asarray(w_dw), np.asarray(w_pw),
        np.asarray(g1), np.asarray(b1), np.asarray(m1), np.asarray(v1),
        np.asarray(g2), np.asarray(b2), np.asarray(m2), np.asarray(v2),
    )
    in_maps = []
    for i in range(N_CORES):
        m = {"x": np.ascontiguousarray(x[i * BS : (i + 1) * BS])}
        m.update(shared)
        in_maps.append(m)

    nc = _get_nc()
    res = run_bass_kernel_spmd(
        nc, in_maps, core_ids=list(range(N_CORES)), trace=_trace, tmpdir=_tmpdir
    )
    y = np.concatenate(
        [res.results[i]["y"].astype(np.float32) for i in range(N_CORES)], axis=0
    )
    if _trace:
        return y, res
    return y


# revision 5
# speedup vs baseline: 1.0236x; 1.0236x over previous
"""Trainium2 Bass kernel for a binary (1w1a) depthwise-separable conv block.

Reference computation (NCHW, B=32, C=CO=512, H=W=56):
    xb  = sign(x)
    y1  = depthwise_conv3x3(xb, sign(w_dw), pad=1)          # per-channel
    z   = sign(y1 * s1 + t1)                                # BN1 + binarize
    y2  = pointwise_conv1x1(z, sign(w_pw))                  # dense 512->512
    out = y2 * s2 + t2                                      # BN2

Sharding: data-parallel over batch, 4 images per core on 8 cores.

Key implementation points:
  - Inputs arrive as bf16 (sign-exact), outputs leave as bf16 and are upcast
    to fp32 on the host: halves DMA traffic both ways.  bf16 output rounding
    is ~0.2% per element, far inside the 2e-2 gate; everything upstream of
    BN2 is exact integer arithmetic.
  - Depthwise conv on TensorE fp8 DoubleRow: 5 accumulating passes per 8-row
    chunk, each contracting 2 taps.  The two DoubleRow "slots" of the moving
    operand are expressed as OVERLAPPING strided access patterns over a
    single zero-padded sign buffer [128, 60, 60] (slot stride 60 = +1 row,
    2 = +2 cols, 0 = duplicate against a zero stationary slot), so no
    shifted copies are materialized.
  - Sign stages are engine-fungible: ScalarE Sign LUT emits +-1; VectorE
    tensor_scalar (psum >_theta) - 0.5 emits +-0.5 (one instruction), with
    the 2x absorbed per-channel into BN1 scale / doubled (+-2) pointwise
    weights.  BN2 evictions run on either engine (ACT Identity with AP
    scale/bias, or DVE mult+add).  The static assignment below balances
    ScalarE and VectorE at ~80us each, under the ~150us TensorE floor.
  - Pointwise conv: fp8 DoubleRow, 2 passes contracting all 512 channels,
    2-chunk PSUM groups (2 banks) so each BN2 eviction covers 896 elements.
"""

import sys

sys.path.insert(0, "/opt/trn_rl_repo")

from contextlib import ExitStack

import ml_dtypes
import numpy as np

import concourse.bass as bass
import concourse.tile as tile
from concourse import mybir
from concourse.ap import AP
from concourse.bass_utils import run_bass_kernel_spmd

N_CORES = 8
B, C, H, W = 32, 512, 56, 56
CO = 512
EPS = 1e-5
BS = B // N_CORES          # images per core
CG = C // 128              # channel groups
ROWS = 8                   # output rows per PSUM chunk (8*56=448 fp32 <= 1 bank)
NCHUNK = H // ROWS         # 7
PH, PW_ = 60, 60           # padded sign-buffer pitch

F32 = mybir.dt.float32
BF16 = mybir.dt.bfloat16
FP8 = mybir.dt.float8e4
DR = mybir.MatmulPerfMode.DoubleRow
NP_FP8 = ml_dtypes.float8_e4m3
NP_BF16 = ml_dtypes.bfloat16

# Engine assignment knobs (must match between host prep and device build).
SIGN_DVE_CGS = frozenset()          # input-sign on DVE (+-0.5) for these cgs
BN1_DVE_CGS = frozenset({1, 2, 3})  # BN1+sign evict on DVE (+-0.5 z)
BN2_ACT_MOD, BN2_ACT_K = 2, 1       # BN2 evict on ACT when ctr % MOD < K


def _legalize_sem_waits(nc, max_waits=1):
    """walrus (CoreV3 codegen) rejects instructions carrying more than one
    sync-wait command.  Tile's kernel-tail drain waits on every outstanding
    semaphore at once; split excess waits onto preceding no-ops on the same
    engine (engines execute their stream in order, so blocking semantics are
    identical)."""
    n_split = 0
    for f in nc.m.functions:
        for bb in f.blocks:
            insts = bb.instructions
            newlist = []
            for inst in insts:
                si = inst.sync_info
                waits = list(si.on_wait) if si is not None else []
                if len(waits) > max_waits:
                    excess, keep = waits[:-max_waits], waits[-max_waits:]
                    for k, w in enumerate(excess):
                        sp = mybir.InstNoOp(name=f"{inst.name}-lgw{k}")
                        sp.engine = inst.engine
                        sp.sync_info = mybir.SyncInfo(on_wait=[w], on_update=[])
                        newlist.append(sp)
                        n_split += 1
                    inst.sync_info = mybir.SyncInfo(
                        on_wait=keep, on_update=list(si.on_update)
                    )
                newlist.append(inst)
            insts[:] = newlist
    return n_split


def build_bass():
    nc = bass.Bass("TRN2", target_bir_lowering=False, debug=False)

    x_d = nc.dram_tensor("x", [BS, C, H, W], BF16, kind="ExternalInput")
    # dw pairs: idx = cg*5 + p; p in 0..2 -> taps (0,p)&(1,p) [slot stride 60];
    # p=3 -> taps (2,0)&(2,2) [slot stride 2]; p=4 -> tap (2,1)&zero [stride 0]
    wdw_d = nc.dram_tensor("wdw", [128, CG * 5, 2, 128], FP8, kind="ExternalInput")
    # pw pairs: idx = zpair*CG + cob; slot j of zpair holds channels
    # (zpair*2+j)*128 ..
    wpw_d = nc.dram_tensor("wpw", [128, 2 * CG, 2, 128], FP8, kind="ExternalInput")
    bn1_d = nc.dram_tensor("bn1", [128, 2 * CG], F32, kind="ExternalInput")
    bn2_d = nc.dram_tensor("bn2", [128, 2 * CG], F32, kind="ExternalInput")
    y_d = nc.dram_tensor("y", [BS, CO, H, W], BF16, kind="ExternalOutput")

    SIGN = mybir.ActivationFunctionType.Sign
    IDENT = mybir.ActivationFunctionType.Identity
    MULT = mybir.AluOpType.mult
    ADD = mybir.AluOpType.add
    GT = mybir.AluOpType.is_gt
    SUB = mybir.AluOpType.subtract

    with tile.TileContext(nc) as tc:
        with ExitStack() as ctx:
            const = ctx.enter_context(tc.tile_pool(name="const", bufs=1))
            xin_pool = ctx.enter_context(tc.tile_pool(name="xin", bufs=8))

            # Prefetch the whole first image before the (bulkier) weight DMAs
            # so the Scalar/Vector/PE pipeline can start ASAP.
            xin_tiles = {}
            t = xin_pool.tile([128, H, W], BF16, tag="xin")
            nc.sync.dma_start(t[:, 0:28, :], x_d.ap()[0, 0:128][:, 0:28, :])
            nc.sync.dma_start(t[:, 28:H, :], x_d.ap()[0, 0:128][:, 28:H, :])
            xin_tiles[(0, 0)] = t

            # interleave the first image's tiles with the per-cg weight
            # slices they unblock, so the first dw block starts ASAP
            wdw_t = const.tile([128, CG * 5, 2, 128], FP8, tag="wdw")
            nc.sync.dma_start(wdw_t[:, 0:5], wdw_d.ap()[:, 0:5])
            bn1_t = const.tile([128, 2 * CG], F32, tag="bn1")
            nc.sync.dma_start(bn1_t[:], bn1_d.ap()[:])
            for pcg in range(1, CG):
                t = xin_pool.tile([128, H, W], BF16, tag="xin")
                nc.sync.dma_start(t[:], x_d.ap()[0, pcg * 128 : (pcg + 1) * 128])
                xin_tiles[(0, pcg)] = t
                nc.sync.dma_start(
                    wdw_t[:, pcg * 5 : (pcg + 1) * 5],
                    wdw_d.ap()[:, pcg * 5 : (pcg + 1) * 5],
                )
            wpw_t = const.tile([128, 2 * CG, 2, 128], FP8, tag="wpw")
            nc.sync.dma_start(wpw_t[:], wpw_d.ap()[:])
            bn2_t = const.tile([128, 2 * CG], F32, tag="bn2")
            nc.sync.dma_start(bn2_t[:], bn2_d.ap()[:])

            # persistent padded sign(x) buffers [128, 60, 60] fp8; only the
            # borders (row 0, row 57, col 0, col 57) need one-time zeros --
            # the interior is rewritten by the per-iteration sign op, and
            # rows/cols beyond 57 are never read.
            xpads = []
            for k in range(3):
                xpa = const.tile([128, PH, PW_], FP8, tag=f"xpad{k}")
                flat = xpa[:].rearrange("p a b -> p (a b)")
                # row 0 (cols 0..59) + row 1 col 0
                nc.vector.memset(flat[:, 0:61], 0)
                # row 57
                nc.vector.memset(flat[:, 57 * 60 : 57 * 60 + 60], 0)
                # rows 1..56: cols 57,58,59 + next row's col 0
                bm = AP(flat.tensor, 60 + 57, [list(flat.ap[0]), [60, 56], [1, 4]])
                nc.vector.memset(bm, 0)
                xpads.append(xpa)

            z_pool = ctx.enter_context(tc.tile_pool(name="z", bufs=4))
            out_pool = ctx.enter_context(tc.tile_pool(name="outb", bufs=4))
            psdw_pool = ctx.enter_context(
                tc.tile_pool(name="psdw", bufs=2, space="PSUM")
            )
            pspw_pool = ctx.enter_context(
                tc.tile_pool(name="pspw", bufs=4, space="PSUM")
            )

            prepared = {}

            def emit_sign(dst, src, cg):
                if cg in SIGN_DVE_CGS:
                    nc.vector.tensor_scalar(dst, src, 0.0, 0.5, GT, SUB)
                else:
                    nc.scalar.activation(dst, src, SIGN)

            def prepare(bp, cgp):
                """Emit the sign stage for iteration (bp,cgp), one iteration
                ahead of the matmul consumer so it never head-of-line-blocks
                the z-evictions that gate PSUM reuse."""
                itp = bp * CG + cgp
                xin = xin_tiles.pop((bp, cgp))
                xpa = xpads[itp % 3]
                if itp == 0:
                    emit_sign(xpa[:, 1:29, 1 : W + 1], xin[:, 0:28, :], cgp)
                    emit_sign(xpa[:, 29 : H + 1, 1 : W + 1], xin[:, 28:H, :], cgp)
                else:
                    emit_sign(xpa[:, 1 : H + 1, 1 : W + 1], xin[:], cgp)
                prepared[(bp, cgp)] = xpa

            def mov(xpa, r0, co, ds):
                """DoubleRow moving AP over the padded buffer: slot stride ds,
                8 rows of 56 from padded row r0, col co."""
                base = xpa[:]
                return AP(
                    base.tensor,
                    base.offset + PH * r0 + co,
                    [list(base.ap[0]), [ds, 2], [PW_, 8], [1, 56]],
                )

            zp_hist = {}
            bn2_ctr = [0]

            def emit_pw(bp):
                zpb = zp_hist.pop(bp)
                for cob in range(CG):
                    outb = out_pool.tile([128, H, W], BF16, tag="outb")
                    for g in range(4):
                        members = [2 * g, 2 * g + 1] if g < 3 else [6]
                        # one 1-bank psum tile per chunk (deep evict
                        # pipelining), but zpair-outer MM order so each
                        # stationary serves len(members) consecutive MMs
                        pps = []
                        for n in members:
                            pp = pspw_pool.tile([128, 512], F32, tag="pspw")
                            pps.append(pp)
                        for zpair in range(2):
                            for s, n in enumerate(members):
                                nc.tensor.matmul(
                                    pps[s][:, 0 : ROWS * W],
                                    wpw_t[:, zpair * CG + cob],
                                    zpb[zpair][:, :, n * ROWS : (n + 1) * ROWS, :],
                                    start=(zpair == 0),
                                    stop=(zpair == 1),
                                    perf_mode=DR,
                                )
                        for s, n in enumerate(members):
                            r0 = n * ROWS
                            oout = outb[:, r0 : r0 + ROWS, :].rearrange(
                                "p r w -> p (r w)"
                            )
                            if bn2_ctr[0] % BN2_ACT_MOD < BN2_ACT_K:
                                nc.scalar.activation(
                                    oout,
                                    pps[s][:, 0 : ROWS * W],
                                    IDENT,
                                    bias=bn2_t[:, cob * 2 + 1 : cob * 2 + 2],
                                    scale=bn2_t[:, cob * 2 : cob * 2 + 1],
                                )
                            else:
                                nc.vector.tensor_scalar(
                                    oout,
                                    pps[s][:, 0 : ROWS * W],
                                    bn2_t[:, cob * 2 : cob * 2 + 1],
                                    bn2_t[:, cob * 2 + 1 : cob * 2 + 2],
                                    MULT,
                                    ADD,
                                )
                            bn2_ctr[0] += 1
                        # stream the output out in halves so the final drain
                        # overlaps compute
                        if g == 1:
                            nc_half = y_d.ap()[bp, cob * 128 : (cob + 1) * 128]
                            nc.sync.dma_start(nc_half[:, 0:32, :], outb[:, 0:32, :])
                    nc.sync.dma_start(
                        y_d.ap()[bp, cob * 128 : (cob + 1) * 128][:, 32:H, :],
                        outb[:, 32:H, :],
                    )

            prepare(0, 0)
            it = 0
            for b in range(BS):
                # prefetch next image's inputs ahead of this image's outputs
                if b + 1 < BS:
                    for pcg in range(CG):
                        t = xin_pool.tile([128, H, W], BF16, tag="xin")
                        nc.sync.dma_start(
                            t[:], x_d.ap()[b + 1, pcg * 128 : (pcg + 1) * 128]
                        )
                        xin_tiles[(b + 1, pcg)] = t
                zp = []
                for _zi in range(2):
                    ztile = z_pool.tile([128, 2, H, W], FP8, tag="z")
                    zp.append(ztile)
                zp_hist[b] = zp
                for cg in range(CG):
                    it += 1
                    # stage the NEXT iteration's sign ahead of this
                    # iteration's matmuls
                    if it < BS * CG:
                        prepare(it // CG, it % CG)
                    xpa = prepared.pop((b, cg))

                    zslot, j = zp[cg // 2], cg % 2
                    # chunk pairs share one 2-bank PSUM tile so the BN1+Sign
                    # eviction reads 2 banks in a single op.  Pass loop is
                    # OUTER so each stationary serves both members
                    # back-to-back.
                    for pg in range(4):
                        members = [2 * pg, 2 * pg + 1] if pg < 3 else [6]
                        m = len(members)
                        ps2 = psdw_pool.tile([128, 2, 512], F32, tag="psdw")
                        # (weight idx, row off, col off, slot stride) per pass
                        passes = [
                            (cg * 5 + 0, 0, 0, PH),
                            (cg * 5 + 1, 0, 1, PH),
                            (cg * 5 + 2, 0, 2, PH),
                            (cg * 5 + 3, 2, 0, 2),
                            (cg * 5 + 4, 2, 1, 0),
                        ]
                        for p, (wi, ro, co, ds) in enumerate(passes):
                            for s, n in enumerate(members):
                                nc.tensor.matmul(
                                    ps2[:, s, 0 : ROWS * W],
                                    wdw_t[:, wi],
                                    mov(xpa, n * ROWS + ro, co, ds),
                                    start=(p == 0),
                                    stop=(p == 4),
                                    perf_mode=DR,
                                )
                        r0 = members[0] * ROWS
                        zout = zslot[:, j, r0 : r0 + ROWS * m, :].rearrange(
                            "p (a r) w -> p a (r w)", a=m
                        )
                        if cg in BN1_DVE_CGS:
                            nc.vector.tensor_scalar(
                                zout,
                                ps2[:, 0:m, 0 : ROWS * W],
                                bn1_t[:, cg * 2 : cg * 2 + 1],
                                0.5,
                                GT,
                                SUB,
                            )
                        else:
                            nc.scalar.activation(
                                zout,
                                ps2[:, 0:m, 0 : ROWS * W],
                                SIGN,
                                bias=bn1_t[:, cg * 2 + 1 : cg * 2 + 2],
                                scale=bn1_t[:, cg * 2 : cg * 2 + 1],
                            )

                    if cg == 0 and b > 0:
                        # previous image's pointwise conv: emitted after
                        # this image's first depthwise block so PE keeps
                        # dense work while the evict engines drain
                        emit_pw(b - 1)

            emit_pw(BS - 1)

    _legalize_sem_waits(nc)
    return nc


_NC_CACHE = None


def _get_nc():
    global _NC_CACHE
    if _NC_CACHE is None:
        _NC_CACHE = build_bass()
    return _NC_CACHE


def make_host_inputs(w_dw, w_pw, g1, b1, m1, v1, g2, b2, m2, v2):
    """Host-side preprocessing shared by all cores (weights/BN constants)."""
    wsign = np.sign(w_dw[:, 0, :, :]).reshape(C, 3, 3).astype(np.float32)

    wdw = np.zeros((128, CG * 5, 2, 128), dtype=NP_FP8)
    idx = np.arange(128)
    for cg in range(CG):
        cs = slice(cg * 128, (cg + 1) * 128)
        for dw in range(3):
            wdw[idx, cg * 5 + dw, 0, idx] = wsign[cs, 0, dw].astype(NP_FP8)
            wdw[idx, cg * 5 + dw, 1, idx] = wsign[cs, 1, dw].astype(NP_FP8)
        # pair 3 (slot stride 2): slot0 = tap (2,0), slot1 = tap (2,2)
        wdw[idx, cg * 5 + 3, 0, idx] = wsign[cs, 2, 0].astype(NP_FP8)
        wdw[idx, cg * 5 + 3, 1, idx] = wsign[cs, 2, 2].astype(NP_FP8)
        # pair 4 (slot stride 0): slot0 = tap (2,1), slot1 stays zero
        wdw[idx, cg * 5 + 4, 0, idx] = wsign[cs, 2, 1].astype(NP_FP8)

    # z scale per input-channel cg: +-0.5 if BN1 evicted on DVE, else +-1;
    # compensated by doubling the pointwise weight for those channels.
    wptT = np.sign(w_pw[:, :, 0, 0]).T.astype(np.float32)  # [c, co]
    for cg in BN1_DVE_CGS:
        wptT[cg * 128 : (cg + 1) * 128, :] *= 2.0
    wpw = np.zeros((128, 2 * CG, 2, 128), dtype=NP_FP8)
    for zpair in range(2):
        for cob in range(CG):
            for j in range(2):
                c0 = (zpair * 2 + j) * 128
                wpw[:, zpair * CG + cob, j, :] = wptT[
                    c0 : c0 + 128, cob * 128 : (cob + 1) * 128
                ].astype(NP_FP8)

    # BN1 constants.  The depthwise psum for cg is alpha*y1 with alpha=0.5
    # when the input sign ran on DVE (+-0.5 xb), else 1.
    s1 = (g1.astype(np.float64) / np.sqrt(v1.astype(np.float64) + EPS)).astype(
        np.float32
    )
    t1 = b1.astype(np.float32) - m1.astype(np.float32) * s1
    bn1 = np.zeros((128, 2 * CG), dtype=np.float32)
    for cg in range(CG):
        cs = slice(cg * 128, (cg + 1) * 128)
        alpha = 0.5 if cg in SIGN_DVE_CGS else 1.0
        if cg in BN1_DVE_CGS:
            # z = (psum > theta') - 0.5 with theta' = -alpha*t1/s1
            s1c = np.maximum(s1[cs], 1e-35)
            theta = -alpha * t1[cs] / s1c
            theta = np.where(
                s1[cs] < 1e-35, np.where(t1[cs] > 0, -1e30, 1e30), theta
            )
            bn1[:, cg * 2] = theta
            bn1[:, cg * 2 + 1] = 0.0
        else:
            bn1[:, cg * 2] = s1[cs] / alpha
            bn1[:, cg * 2 + 1] = t1[cs]

    s2 = (g2.astype(np.float64) / np.sqrt(v2.astype(np.float64) + EPS)).astype(
        np.float32
    )
    t2 = b2.astype(np.float32) - m2.astype(np.float32) * s2
    bn2 = np.zeros((128, 2 * CG), dtype=np.float32)
    for cg in range(CG):
        bn2[:, cg * 2] = s2[cg * 128 : (cg + 1) * 128]
        bn2[:, cg * 2 + 1] = t2[cg * 128 : (cg + 1) * 128]

    return {"wdw": wdw, "wpw": wpw, "bn1": bn1, "bn2": bn2}


def kernel(x, w_dw, w_pw, g1, b1, m1, v1, g2, b2, m2, v2, _trace=False, _tmpdir=None):
    x = np.asarray(x, dtype=np.float32).astype(NP_BF16)
    shared = make_host_inputs(
        np.asarray(w_dw), np.asarray(w_pw),
        np.asarray(g1), np.asarray(b1), np.asarray(m1), np.asarray(v1),
        np.asarray(g2), np.asarray(b2), np.asarray(m2), np.asarray(v2),
    )
    in_maps = []
    for i in range(N_CORES):
        m = {"x": np.ascontiguousarray(x[i * BS : (i + 1) * BS])}
        m.update(shared)
        in_maps.append(m)

    nc = _get_nc()
    res = run_bass_kernel_spmd(
        nc, in_maps, core_ids=list(range(N_CORES)), trace=_trace, tmpdir=_tmpdir
    )
    y = np.concatenate(
        [res.results[i]["y"].astype(np.float32) for i in range(N_CORES)], axis=0
    )
    if _trace:
        return y, res
    return y


# revision 6
# speedup vs baseline: 1.0575x; 1.0331x over previous
"""Baseline kernel (restored from session start) - for cadence comparison."""

import sys

sys.path.insert(0, "/opt/trn_rl_repo")

from contextlib import ExitStack

import ml_dtypes
import numpy as np

import concourse.bass as bass
import concourse.tile as tile
from concourse import mybir
from concourse.bass_utils import run_bass_kernel_spmd

N_CORES = 8
B, C, H, W = 32, 512, 56, 56
CO = 512
EPS = 1e-5
BS = B // N_CORES          # images per core
CG = C // 128              # channel groups
ROWS = 8                   # output rows per PSUM chunk (8*56=448 fp32 <= 1 bank)
NCHUNK = H // ROWS         # 7
PH, PW_ = 60, 60           # padded buffer pitch

F32 = mybir.dt.float32
FP8 = mybir.dt.float8e4
DR = mybir.MatmulPerfMode.DoubleRow
NP_FP8 = ml_dtypes.float8_e4m3


def _legalize_sem_waits(nc, max_waits=1):
    n_split = 0
    for f in nc.m.functions:
        for bb in f.blocks:
            insts = bb.instructions
            newlist = []
            for inst in insts:
                si = inst.sync_info
                waits = list(si.on_wait) if si is not None else []
                if len(waits) > max_waits:
                    excess, keep = waits[:-max_waits], waits[-max_waits:]
                    for k, w in enumerate(excess):
                        sp = mybir.InstNoOp(name=f"{inst.name}-lgw{k}")
                        sp.engine = inst.engine
                        sp.sync_info = mybir.SyncInfo(on_wait=[w], on_update=[])
                        newlist.append(sp)
                        n_split += 1
                    inst.sync_info = mybir.SyncInfo(
                        on_wait=keep, on_update=list(si.on_update)
                    )
                newlist.append(inst)
            insts[:] = newlist
    return n_split


def build_bass():
    nc = bass.Bass("TRN2", target_bir_lowering=False, debug=False)

    x_d = nc.dram_tensor("x", [BS, C, H, W], F32, kind="ExternalInput")
    wdw_d = nc.dram_tensor("wdw", [128, CG * 5, 2, 128], FP8, kind="ExternalInput")
    wpw_d = nc.dram_tensor("wpw", [128, 2 * CG, 2, 128], FP8, kind="ExternalInput")
    bn1_d = nc.dram_tensor("bn1", [128, 2 * CG], F32, kind="ExternalInput")
    bn2_d = nc.dram_tensor("bn2", [128, 2 * CG], F32, kind="ExternalInput")
    y_d = nc.dram_tensor("y", [BS, CO, H, W], F32, kind="ExternalOutput")

    SIGN = mybir.ActivationFunctionType.Sign
    MULT = mybir.AluOpType.mult
    ADD = mybir.AluOpType.add

    with tile.TileContext(nc) as tc:
        with ExitStack() as ctx:
            const = ctx.enter_context(tc.tile_pool(name="const", bufs=1))
            xin_pool = ctx.enter_context(tc.tile_pool(name="xin", bufs=6))

            xin_tiles = {}
            t = xin_pool.tile([128, H, W], F32, tag="xin")
            nc.sync.dma_start(t[:, 0:28, :], x_d.ap()[0, 0:128][:, 0:28, :])
            nc.sync.dma_start(t[:, 28:H, :], x_d.ap()[0, 0:128][:, 28:H, :])
            xin_tiles[(0, 0)] = t

            wdw_t = const.tile([128, CG * 5, 2, 128], FP8, tag="wdw")
            for wcg in range(CG):
                nc.sync.dma_start(
                    wdw_t[:, wcg * 5 : (wcg + 1) * 5],
                    wdw_d.ap()[:, wcg * 5 : (wcg + 1) * 5],
                )
            for pcg in range(1, CG):
                t = xin_pool.tile([128, H, W], F32, tag="xin")
                nc.sync.dma_start(t[:], x_d.ap()[0, pcg * 128 : (pcg + 1) * 128])
                xin_tiles[(0, pcg)] = t
            wpw_t = const.tile([128, 2 * CG, 2, 128], FP8, tag="wpw")
            nc.sync.dma_start(wpw_t[:], wpw_d.ap()[:])
            bn1_t = const.tile([128, 2 * CG], F32, tag="bn1")
            nc.sync.dma_start(bn1_t[:], bn1_d.ap()[:])
            bn2_t = const.tile([128, 2 * CG], F32, tag="bn2")
            nc.sync.dma_start(bn2_t[:], bn2_d.ap()[:])

            xpads = []
            for k in range(3):
                xpa = const.tile([128, 2, PH, PW_], FP8, tag=f"xpada{k}")
                xp32 = xpa[:].rearrange("p a b c -> p (a b c)").bitcast(
                    mybir.dt.uint32
                )
                nc.vector.memset(xp32, 0)
                xpb = const.tile([128, 2, PH, PW_], FP8, tag=f"xpadb{k}")
                xpads.append((xpa, xpb))

            z_pool = ctx.enter_context(tc.tile_pool(name="z", bufs=4))
            out_pool = ctx.enter_context(tc.tile_pool(name="outb", bufs=2))
            psdw_pool = ctx.enter_context(
                tc.tile_pool(name="psdw", bufs=2, space="PSUM")
            )
            pspw_pool = ctx.enter_context(
                tc.tile_pool(name="pspw", bufs=4, space="PSUM")
            )

            prepared = {}

            def prepare(bp, cgp):
                itp = bp * CG + cgp
                xin = xin_tiles.pop((bp, cgp))
                xpa, xpb = xpads[itp % 3]
                if itp == 0:
                    nc.scalar.activation(
                        xpa[:, 0, 1:29, 1 : W + 1], xin[:, 0:28, :], SIGN
                    )
                    nc.scalar.activation(
                        xpa[:, 0, 29 : H + 1, 1 : W + 1], xin[:, 28:H, :], SIGN
                    )
                else:
                    nc.scalar.activation(
                        xpa[:, 0, 1 : H + 1, 1 : W + 1], xin[:], SIGN
                    )
                fa = xpa[:].rearrange("p a b c -> p (a b c)").bitcast(
                    mybir.dt.bfloat16
                )
                fb = xpb[:].rearrange("p a b c -> p (a b c)").bitcast(
                    mybir.dt.bfloat16
                )
                nc.vector.tensor_copy(fa[:, 1800:3540], fa[:, 30:1770])
                nc.vector.tensor_copy(fb[:, 0:1740], fa[:, 0:1740])
                nc.vector.tensor_copy(fb[:, 1800:3540], fa[:, 1:1741])
                prepared[(bp, cgp)] = (xpa, xpb)

            zp_hist = {}

            def emit_pw(bp):
                zpb = zp_hist.pop(bp)
                for cob in range(CG):
                    outb = out_pool.tile([128, H, W], F32, tag="outb")
                    for n in range(NCHUNK):
                        pp = pspw_pool.tile([128, 512], F32, tag="pspw")
                        r0 = n * ROWS
                        for zpair in range(2):
                            nc.tensor.matmul(
                                pp[:, 0 : ROWS * W],
                                wpw_t[:, zpair * CG + cob],
                                zpb[zpair][:, :, r0 : r0 + ROWS, :],
                                start=(zpair == 0),
                                stop=(zpair == 1),
                                perf_mode=DR,
                            )
                        oout = outb[:, r0 : r0 + ROWS, :].rearrange(
                            "p r w -> p (r w)"
                        )
                        nc.vector.tensor_scalar(
                            oout,
                            pp[:, 0 : ROWS * W],
                            bn2_t[:, cob * 2 : cob * 2 + 1],
                            bn2_t[:, cob * 2 + 1 : cob * 2 + 2],
                            MULT,
                            ADD,
                        )
                        if n == 3:
                            nc_half = y_d.ap()[bp, cob * 128 : (cob + 1) * 128]
                            nc.sync.dma_start(nc_half[:, 0:32, :], outb[:, 0:32, :])
                    nc.sync.dma_start(
                        y_d.ap()[bp, cob * 128 : (cob + 1) * 128][:, 32:H, :],
                        outb[:, 32:H, :],
                    )

            prepare(0, 0)
            it = 0
            for b in range(BS):
                if b + 1 < BS:
                    for pcg in range(CG):
                        t = xin_pool.tile([128, H, W], F32, tag="xin")
                        nc.sync.dma_start(
                            t[:], x_d.ap()[b + 1, pcg * 128 : (pcg + 1) * 128]
                        )
                        xin_tiles[(b + 1, pcg)] = t
                zp = []
                for _zi in range(2):
                    ztile = z_pool.tile([128, 2, H, W], FP8, tag="z")
                    zp.append(ztile)
                zp_hist[b] = zp
                for cg in range(CG):
                    it += 1
                    if it < BS * CG:
                        prepare(it // CG, it % CG)
                    xpa, xpb = prepared.pop((b, cg))

                    zslot, j = zp[cg // 2], cg % 2
                    for pg in range(4):
                        members = [2 * pg, 2 * pg + 1] if pg < 3 else [6]
                        ps2 = psdw_pool.tile([128, 2, 512], F32, tag="psdw")
                        passes = [
                            (cg * 5 + 0, xpa, 0, 0),
                            (cg * 5 + 1, xpa, 0, 1),
                            (cg * 5 + 2, xpa, 0, 2),
                            (cg * 5 + 3, xpb, 2, 0),
                            (cg * 5 + 4, xpa, 2, 1),
                        ]
                        for p, (wi, buf, ro, co) in enumerate(passes):
                            for s, n in enumerate(members):
                                r0 = n * ROWS + ro
                                nc.tensor.matmul(
                                    ps2[:, s, 0 : ROWS * W],
                                    wdw_t[:, wi],
                                    buf[:, :, r0 : r0 + ROWS, co : co + W],
                                    start=(p == 0),
                                    stop=(p == 4),
                                    perf_mode=DR,
                                )
                        r0 = members[0] * ROWS
                        nrows = ROWS * len(members)
                        zout = zslot[:, j, r0 : r0 + nrows, :].rearrange(
                            "p (a r) w -> p a (r w)", a=len(members)
                        )
                        nc.scalar.activation(
                            zout,
                            ps2[:, 0 : len(members), 0 : ROWS * W],
                            SIGN,
                            bias=bn1_t[:, cg * 2 + 1 : cg * 2 + 2],
                            scale=bn1_t[:, cg * 2 : cg * 2 + 1],
                        )

                    if cg == 0 and b > 0:
                        emit_pw(b - 1)

            emit_pw(BS - 1)

    _legalize_sem_waits(nc)
    return nc


_NC_CACHE = None


def _get_nc():
    global _NC_CACHE
    if _NC_CACHE is None:
        _NC_CACHE = build_bass()
    return _NC_CACHE


def make_host_inputs(w_dw, w_pw, g1, b1, m1, v1, g2, b2, m2, v2):
    wsign = np.sign(w_dw[:, 0, :, :]).reshape(C, 3, 3).astype(np.float32)

    wdw = np.zeros((128, CG * 5, 2, 128), dtype=NP_FP8)
    idx = np.arange(128)
    for cg in range(CG):
        cs = slice(cg * 128, (cg + 1) * 128)
        for dw in range(3):
            wdw[idx, cg * 5 + dw, 0, idx] = wsign[cs, 0, dw].astype(NP_FP8)
            wdw[idx, cg * 5 + dw, 1, idx] = wsign[cs, 1, dw].astype(NP_FP8)
        wdw[idx, cg * 5 + 3, 0, idx] = wsign[cs, 2, 0].astype(NP_FP8)
        wdw[idx, cg * 5 + 3, 1, idx] = wsign[cs, 2, 2].astype(NP_FP8)
        wdw[idx, cg * 5 + 4, 0, idx] = wsign[cs, 2, 1].astype(NP_FP8)

    wptT = np.sign(w_pw[:, :, 0, 0]).T.astype(np.float32)
    wpw = np.zeros((128, 2 * CG, 2, 128), dtype=NP_FP8)
    for zpair in range(2):
        for cob in range(CG):
            for j in range(2):
                c0 = (zpair * 2 + j) * 128
                wpw[:, zpair * CG + cob, j, :] = wptT[
                    c0 : c0 + 128, cob * 128 : (cob + 1) * 128
                ].astype(NP_FP8)

    def bn_consts(g, bta, m, v):
        s = (g.astype(np.float64) / np.sqrt(v.astype(np.float64) + EPS)).astype(
            np.float32
        )
        t = bta.astype(np.float32) - m.astype(np.float32) * s
        out = np.zeros((128, 2 * CG), dtype=np.float32)
        for cg in range(CG):
            out[:, cg * 2] = s[cg * 128 : (cg + 1) * 128]
            out[:, cg * 2 + 1] = t[cg * 128 : (cg + 1) * 128]
        return out

    return {
        "wdw": wdw,
        "wpw": wpw,
        "bn1": bn_consts(g1, b1, m1, v1),
        "bn2": bn_consts(g2, b2, m2, v2),
    }


def kernel(x, w_dw, w_pw, g1, b1, m1, v1, g2, b2, m2, v2, _trace=False, _tmpdir=None):
    x = np.asarray(x, dtype=np.float32)
    shared = make_host_inputs(
        np.asarray(w_dw), np.asarray(w_pw),
        np.asarray(g1), np.asarray(b1), np.asarray(m1), np.asarray(v1),
        np.asarray(g2), np.asarray(b2), np.asarray(m2), np.asarray(v2),
    )
    in_maps = []
    for i in range(N_CORES):
        m = {"x": np.ascontiguousarray(x[i * BS : (i + 1) * BS])}
        m.update(shared)
        in_maps.append(m)

    nc = _get_nc()
    res = run_bass_kernel_spmd(
        nc, in_maps, core_ids=list(range(N_CORES)), trace=_trace, tmpdir=_tmpdir
    )
    y = np.concatenate([res.results[i]["y"] for i in range(N_CORES)], axis=0)
    if _trace:
        return y, res
    return y


# revision 11
# speedup vs baseline: 1.2081x; 1.1424x over previous
"""Trainium2 Bass kernel for a binary (1w1a) depthwise-separable conv block.

Reference computation (NCHW, B=32, C=CO=512, H=W=56):
    xb  = sign(x)
    y1  = depthwise_conv3x3(xb, sign(w_dw), pad=1)          # per-channel
    z   = sign(y1 * s1 + t1)                                # BN1 + binarize
    y2  = pointwise_conv1x1(z, sign(w_pw))                  # dense 512->512
    out = y2 * s2 + t2                                      # BN2

Sharding: data-parallel over batch, 4 images per core on 8 cores.

Key implementation points:
  - Inputs arrive as bf16 (sign-exact), outputs leave as bf16 and are upcast
    to fp32 on the host: halves DMA traffic both ways.  bf16 output rounding
    is ~0.2% per element, far inside the 2e-2 gate; everything upstream of
    BN2 is exact integer arithmetic.
  - Depthwise conv on TensorE fp8 DoubleRow: 5 accumulating passes per 8-row
    chunk, each contracting 2 taps.  The two DoubleRow "slots" of the moving
    operand are expressed as OVERLAPPING strided access patterns over a
    single zero-padded sign buffer [128, 60, 60] (slot stride 60 = +1 row,
    2 = +2 cols, 0 = duplicate against a zero stationary slot), so no
    shifted copies are materialized.
  - Sign stages are engine-fungible: ScalarE Sign LUT emits +-1; VectorE
    tensor_scalar (psum >_theta) - 0.5 emits +-0.5 (one instruction), with
    the 2x absorbed per-channel into BN1 scale / doubled (+-2) pointwise
    weights.  BN2 evictions run on either engine (ACT Identity with AP
    scale/bias, or DVE mult+add).  The static assignment below balances
    ScalarE and VectorE at ~80us each, under the ~150us TensorE floor.
  - Pointwise conv: fp8 DoubleRow, 2 passes contracting all 512 channels,
    2-chunk PSUM groups (2 banks) so each BN2 eviction covers 896 elements.
"""

import sys

sys.path.insert(0, "/opt/trn_rl_repo")

from contextlib import ExitStack

import ml_dtypes
import numpy as np

import concourse.bass as bass
import concourse.tile as tile
from concourse import mybir
from concourse.ap import AP
from concourse.bass_utils import run_bass_kernel_spmd

N_CORES = 8
B, C, H, W = 32, 512, 56, 56
CO = 512
EPS = 1e-5
BS = B // N_CORES          # images per core
CG = C // 128              # channel groups
ROWS = 8                   # output rows per PSUM chunk (8*56=448 fp32 <= 1 bank)
NCHUNK = H // ROWS         # 7
PH, PW_ = 60, 60           # padded sign-buffer pitch

F32 = mybir.dt.float32
BF16 = mybir.dt.bfloat16
FP8 = mybir.dt.float8e4
DR = mybir.MatmulPerfMode.DoubleRow
NP_FP8 = ml_dtypes.float8_e4m3
NP_BF16 = ml_dtypes.bfloat16

# Engine assignment knobs (must match between host prep and device build).
SIGN_DVE_CGS = frozenset()          # input-sign on DVE (+-0.5) for these cgs
BN1_DVE_CGS = frozenset({1, 2, 3})  # BN1+sign evict on DVE (+-0.5 z)
BN2_ACT_MOD, BN2_ACT_K = 2, 1       # BN2 evict on ACT when ctr % MOD < K
COPY_CGS = frozenset({2, 3})        # A/B: cgs using materialized shift copies


def _legalize_sem_waits(nc, max_waits=1):
    """walrus (CoreV3 codegen) rejects instructions carrying more than one
    sync-wait command.  Tile's kernel-tail drain waits on every outstanding
    semaphore at once; split excess waits onto preceding no-ops on the same
    engine (engines execute their stream in order, so blocking semantics are
    identical)."""
    n_split = 0
    for f in nc.m.functions:
        for bb in f.blocks:
            insts = bb.instructions
            newlist = []
            for inst in insts:
                si = inst.sync_info
                waits = list(si.on_wait) if si is not None else []
                if len(waits) > max_waits:
                    excess, keep = waits[:-max_waits], waits[-max_waits:]
                    for k, w in enumerate(excess):
                        sp = mybir.InstNoOp(name=f"{inst.name}-lgw{k}")
                        sp.engine = inst.engine
                        sp.sync_info = mybir.SyncInfo(on_wait=[w], on_update=[])
                        newlist.append(sp)
                        n_split += 1
                    inst.sync_info = mybir.SyncInfo(
                        on_wait=keep, on_update=list(si.on_update)
                    )
                newlist.append(inst)
            insts[:] = newlist
    return n_split


def build_bass():
    nc = bass.Bass("TRN2", target_bir_lowering=False, debug=False)

    x_d = nc.dram_tensor("x", [BS, C, H, W], BF16, kind="ExternalInput")
    # dw pairs: idx = cg*5 + p; p in 0..2 -> taps (0,p)&(1,p) [slot stride 60];
    # p=3 -> taps (2,0)&(2,2) [slot stride 2]; p=4 -> tap (2,1)&zero [stride 0]
    wdw_d = nc.dram_tensor("wdw", [128, CG * 5, 2, 128], FP8, kind="ExternalInput")
    # pw pairs: idx = zpair*CG + cob; slot j of zpair holds channels
    # (zpair*2+j)*128 ..
    wpw_d = nc.dram_tensor("wpw", [128, 2 * CG, 2, 128], FP8, kind="ExternalInput")
    bn1_d = nc.dram_tensor("bn1", [128, 2 * CG], F32, kind="ExternalInput")
    bn2_d = nc.dram_tensor("bn2", [128, 2 * CG], F32, kind="ExternalInput")
    y_d = nc.dram_tensor("y", [BS, CO, H, W], BF16, kind="ExternalOutput")

    SIGN = mybir.ActivationFunctionType.Sign
    IDENT = mybir.ActivationFunctionType.Identity
    MULT = mybir.AluOpType.mult
    ADD = mybir.AluOpType.add
    GT = mybir.AluOpType.is_gt
    SUB = mybir.AluOpType.subtract

    with tile.TileContext(nc) as tc:
        with ExitStack() as ctx:
            const = ctx.enter_context(tc.tile_pool(name="const", bufs=1))
            xin_pool = ctx.enter_context(tc.tile_pool(name="xin", bufs=8))

            # Prefetch the whole first image before the (bulkier) weight DMAs
            # so the Scalar/Vector/PE pipeline can start ASAP.
            xin_tiles = {}
            t = xin_pool.tile([128, H, W], BF16, tag="xin")
            nc.sync.dma_start(t[:, 0:28, :], x_d.ap()[0, 0:128][:, 0:28, :])
            nc.sync.dma_start(t[:, 28:H, :], x_d.ap()[0, 0:128][:, 28:H, :])
            xin_tiles[(0, 0)] = t

            # interleave the first image's tiles with the per-cg weight
            # slices they unblock, so the first dw block starts ASAP
            wdw_t = const.tile([128, CG * 5, 2, 128], FP8, tag="wdw")
            nc.sync.dma_start(wdw_t[:, 0:5], wdw_d.ap()[:, 0:5])
            bn1_t = const.tile([128, 2 * CG], F32, tag="bn1")
            nc.sync.dma_start(bn1_t[:], bn1_d.ap()[:])
            for pcg in range(1, CG):
                t = xin_pool.tile([128, H, W], BF16, tag="xin")
                nc.sync.dma_start(t[:], x_d.ap()[0, pcg * 128 : (pcg + 1) * 128])
                xin_tiles[(0, pcg)] = t
                nc.sync.dma_start(
                    wdw_t[:, pcg * 5 : (pcg + 1) * 5],
                    wdw_d.ap()[:, pcg * 5 : (pcg + 1) * 5],
                )
            wpw_t = const.tile([128, 2 * CG, 2, 128], FP8, tag="wpw")
            nc.sync.dma_start(wpw_t[:], wpw_d.ap()[:])
            bn2_t = const.tile([128, 2 * CG], F32, tag="bn2")
            nc.sync.dma_start(bn2_t[:], bn2_d.ap()[:])

            # persistent padded sign(x) buffers [128, 60, 60] fp8; only the
            # borders (row 0, row 57, col 0, col 57) need one-time zeros --
            # the interior is rewritten by the per-iteration sign op, and
            # rows/cols beyond 57 are never read.
            xpads = []
            for k in range(3):
                xpa = const.tile([128, 2, PH, PW_], FP8, tag=f"xpad{k}")
                xp32 = xpa[:].rearrange("p a b c -> p (a b c)").bitcast(
                    mybir.dt.uint32
                )
                nc.vector.memset(xp32, 0)
                xpb = const.tile([128, 2, PH, PW_], FP8, tag=f"xpadb{k}")
                xpads.append((xpa, xpb))

            z_pool = ctx.enter_context(tc.tile_pool(name="z", bufs=4))
            out_pool = ctx.enter_context(tc.tile_pool(name="outb", bufs=4))
            psdw_pool = ctx.enter_context(
                tc.tile_pool(name="psdw", bufs=2, space="PSUM")
            )
            pspw_pool = ctx.enter_context(
                tc.tile_pool(name="pspw", bufs=4, space="PSUM")
            )

            prepared = {}

            def emit_sign(dst, src, cg):
                if cg in SIGN_DVE_CGS:
                    nc.vector.tensor_scalar(dst, src, 0.0, 0.5, GT, SUB)
                else:
                    nc.scalar.activation(dst, src, SIGN)

            def prepare(bp, cgp):
                """Emit the sign stage for iteration (bp,cgp), one iteration
                ahead of the matmul consumer so it never head-of-line-blocks
                the z-evictions that gate PSUM reuse."""
                itp = bp * CG + cgp
                xin = xin_tiles.pop((bp, cgp))
                xpa, xpb = xpads[itp % 3]
                if itp == 0:
                    emit_sign(xpa[:, 0, 1:29, 1 : W + 1], xin[:, 0:28, :], cgp)
                    emit_sign(
                        xpa[:, 0, 29 : H + 1, 1 : W + 1], xin[:, 28:H, :], cgp
                    )
                else:
                    emit_sign(xpa[:, 0, 1 : H + 1, 1 : W + 1], xin[:], cgp)
                if cgp in COPY_CGS:
                    fa = xpa[:].rearrange("p a b c -> p (a b c)").bitcast(
                        mybir.dt.bfloat16
                    )
                    fb = xpb[:].rearrange("p a b c -> p (a b c)").bitcast(
                        mybir.dt.bfloat16
                    )
                    nc.vector.tensor_copy(fa[:, 1800:3540], fa[:, 30:1770])
                    nc.vector.tensor_copy(fb[:, 0:1740], fa[:, 0:1740])
                    nc.vector.tensor_copy(fb[:, 1800:3540], fa[:, 1:1741])
                prepared[(bp, cgp)] = (xpa, xpb)

            def mov(buf, r0, co, ds):
                """DoubleRow moving AP over the padded buffer: slot stride ds,
                8 rows of 56 from padded row r0, col co."""
                base = buf[:]
                return AP(
                    base.tensor,
                    base.offset + PH * r0 + co,
                    [list(base.ap[0]), [ds, 2], [PW_, 8], [1, 56]],
                )

            zp_hist = {}
            bn2_ctr = [0]

            def emit_pw(bp):
                zpb = zp_hist.pop(bp)
                for cob in range(CG):
                    outb = out_pool.tile([128, H, W], BF16, tag="outb")
                    for g in range(4):
                        members = [2 * g, 2 * g + 1] if g < 3 else [6]
                        # one 1-bank psum tile per chunk (deep evict
                        # pipelining), but zpair-outer MM order so each
                        # stationary serves len(members) consecutive MMs
                        pps = []
                        for n in members:
                            pp = pspw_pool.tile([128, 512], F32, tag="pspw")
                            pps.append(pp)
                        for zpair in range(2):
                            for s, n in enumerate(members):
                                nc.tensor.matmul(
                                    pps[s][:, 0 : ROWS * W],
                                    wpw_t[:, zpair * CG + cob],
                                    zpb[zpair][:, :, n * ROWS : (n + 1) * ROWS, :],
                                    start=(zpair == 0),
                                    stop=(zpair == 1),
                                    perf_mode=DR,
                                )
                        for s, n in enumerate(members):
                            r0 = n * ROWS
                            oout = outb[:, r0 : r0 + ROWS, :].rearrange(
                                "p r w -> p (r w)"
                            )
                            if bn2_ctr[0] % BN2_ACT_MOD < BN2_ACT_K:
                                nc.scalar.activation(
                                    oout,
                                    pps[s][:, 0 : ROWS * W],
                                    IDENT,
                                    bias=bn2_t[:, cob * 2 + 1 : cob * 2 + 2],
                                    scale=bn2_t[:, cob * 2 : cob * 2 + 1],
                                )
                            else:
                                nc.vector.tensor_scalar(
                                    oout,
                                    pps[s][:, 0 : ROWS * W],
                                    bn2_t[:, cob * 2 : cob * 2 + 1],
                                    bn2_t[:, cob * 2 + 1 : cob * 2 + 2],
                                    MULT,
                                    ADD,
                                )
                            bn2_ctr[0] += 1
                        # stream the output out in halves so the final drain
                        # overlaps compute
                        if g == 1:
                            nc_half = y_d.ap()[bp, cob * 128 : (cob + 1) * 128]
                            nc.sync.dma_start(nc_half[:, 0:32, :], outb[:, 0:32, :])
                    nc.sync.dma_start(
                        y_d.ap()[bp, cob * 128 : (cob + 1) * 128][:, 32:H, :],
                        outb[:, 32:H, :],
                    )

            prepare(0, 0)
            it = 0
            for b in range(BS):
                # prefetch next image's inputs ahead of this image's outputs
                if b + 1 < BS:
                    for pcg in range(CG):
                        t = xin_pool.tile([128, H, W], BF16, tag="xin")
                        nc.sync.dma_start(
                            t[:], x_d.ap()[b + 1, pcg * 128 : (pcg + 1) * 128]
                        )
                        xin_tiles[(b + 1, pcg)] = t
                zp = []
                for _zi in range(2):
                    ztile = z_pool.tile([128, 2, H, W], FP8, tag="z")
                    zp.append(ztile)
                zp_hist[b] = zp
                for cg in range(CG):
                    it += 1
                    # stage the NEXT iteration's sign ahead of this
                    # iteration's matmuls
                    if it < BS * CG:
                        prepare(it // CG, it % CG)
                    xpa, xpb = prepared.pop((b, cg))

                    zslot, j = zp[cg // 2], cg % 2
                    # chunk pairs share one 2-bank PSUM tile so the BN1+Sign
                    # eviction reads 2 banks in a single op.  Pass loop is
                    # OUTER so each stationary serves both members
                    # back-to-back.
                    for pg in range(4):
                        members = [2 * pg, 2 * pg + 1] if pg < 3 else [6]
                        m = len(members)
                        ps2 = psdw_pool.tile([128, 2, 512], F32, tag="psdw")
                        # (weight idx, buf, row off, col off, slot stride)
                        if cg in COPY_CGS:
                            passes = [
                                (cg * 5 + 0, xpa, 0, 0, 3600),
                                (cg * 5 + 1, xpa, 0, 1, 3600),
                                (cg * 5 + 2, xpa, 0, 2, 3600),
                                (cg * 5 + 3, xpb, 2, 0, 3600),
                                (cg * 5 + 4, xpa, 2, 1, 3600),
                            ]
                        else:
                            passes = [
                                (cg * 5 + 0, xpa, 0, 0, PH),
                                (cg * 5 + 1, xpa, 0, 1, PH),
                                (cg * 5 + 2, xpa, 0, 2, PH),
                                (cg * 5 + 3, xpa, 2, 0, 2),
                                (cg * 5 + 4, xpa, 2, 1, 0),
                            ]
                        for p, (wi, buf, ro, co, ds) in enumerate(passes):
                            for s, n in enumerate(members):
                                nc.tensor.matmul(
                                    ps2[:, s, 0 : ROWS * W],
                                    wdw_t[:, wi],
                                    mov(buf, n * ROWS + ro, co, ds),
                                    start=(p == 0),
                                    stop=(p == 4),
                                    perf_mode=DR,
                                )
                        r0 = members[0] * ROWS
                        zout = zslot[:, j, r0 : r0 + ROWS * m, :].rearrange(
                            "p (a r) w -> p a (r w)", a=m
                        )
                        if cg in BN1_DVE_CGS:
                            nc.vector.tensor_scalar(
                                zout,
                                ps2[:, 0:m, 0 : ROWS * W],
                                bn1_t[:, cg * 2 : cg * 2 + 1],
                                0.5,
                                GT,
                                SUB,
                            )
                        else:
                            nc.scalar.activation(
                                zout,
                                ps2[:, 0:m, 0 : ROWS * W],
                                SIGN,
                                bias=bn1_t[:, cg * 2 + 1 : cg * 2 + 2],
                                scale=bn1_t[:, cg * 2 : cg * 2 + 1],
                            )

                    if cg == 0 and b > 0:
                        # previous image's pointwise conv: emitted after
                        # this image's first depthwise block so PE keeps
                        # dense work while the evict engines drain
                        emit_pw(b - 1)

            emit_pw(BS - 1)

    _legalize_sem_waits(nc)
    return nc


_NC_CACHE = None


def _get_nc():
    global _NC_CACHE
    if _NC_CACHE is None:
        _NC_CACHE = build_bass()
    return _NC_CACHE


def make_host_inputs(w_dw, w_pw, g1, b1, m1, v1, g2, b2, m2, v2):
    """Host-side preprocessing shared by all cores (weights/BN constants)."""
    wsign = np.sign(w_dw[:, 0, :, :]).reshape(C, 3, 3).astype(np.float32)

    wdw = np.zeros((128, CG * 5, 2, 128), dtype=NP_FP8)
    idx = np.arange(128)
    for cg in range(CG):
        cs = slice(cg * 128, (cg + 1) * 128)
        for dw in range(3):
            wdw[idx, cg * 5 + dw, 0, idx] = wsign[cs, 0, dw].astype(NP_FP8)
            wdw[idx, cg * 5 + dw, 1, idx] = wsign[cs, 1, dw].astype(NP_FP8)
        # pair 3 (slot stride 2): slot0 = tap (2,0), slot1 = tap (2,2)
        wdw[idx, cg * 5 + 3, 0, idx] = wsign[cs, 2, 0].astype(NP_FP8)
        wdw[idx, cg * 5 + 3, 1, idx] = wsign[cs, 2, 2].astype(NP_FP8)
        # pair 4 (slot stride 0): slot0 = tap (2,1), slot1 stays zero
        wdw[idx, cg * 5 + 4, 0, idx] = wsign[cs, 2, 1].astype(NP_FP8)

    # z scale per input-channel cg: +-0.5 if BN1 evicted on DVE, else +-1;
    # compensated by doubling the pointwise weight for those channels.
    wptT = np.sign(w_pw[:, :, 0, 0]).T.astype(np.float32)  # [c, co]
    for cg in BN1_DVE_CGS:
        wptT[cg * 128 : (cg + 1) * 128, :] *= 2.0
    wpw = np.zeros((128, 2 * CG, 2, 128), dtype=NP_FP8)
    for zpair in range(2):
        for cob in range(CG):
            for j in range(2):
                c0 = (zpair * 2 + j) * 128
                wpw[:, zpair * CG + cob, j, :] = wptT[
                    c0 : c0 + 128, cob * 128 : (cob + 1) * 128
                ].astype(NP_FP8)

    # BN1 constants.  The depthwise psum for cg is alpha*y1 with alpha=0.5
    # when the input sign ran on DVE (+-0.5 xb), else 1.
    s1 = (g1.astype(np.float64) / np.sqrt(v1.astype(np.float64) + EPS)).astype(
        np.float32
    )
    t1 = b1.astype(np.float32) - m1.astype(np.float32) * s1
    bn1 = np.zeros((128, 2 * CG), dtype=np.float32)
    for cg in range(CG):
        cs = slice(cg * 128, (cg + 1) * 128)
        alpha = 0.5 if cg in SIGN_DVE_CGS else 1.0
        if cg in BN1_DVE_CGS:
            # z = (psum > theta') - 0.5 with theta' = -alpha*t1/s1
            s1c = np.maximum(s1[cs], 1e-35)
            theta = -alpha * t1[cs] / s1c
            theta = np.where(
                s1[cs] < 1e-35, np.where(t1[cs] > 0, -1e30, 1e30), theta
            )
            bn1[:, cg * 2] = theta
            bn1[:, cg * 2 + 1] = 0.0
        else:
            bn1[:, cg * 2] = s1[cs] / alpha
            bn1[:, cg * 2 + 1] = t1[cs]

    s2 = (g2.astype(np.float64) / np.sqrt(v2.astype(np.float64) + EPS)).astype(
        np.float32
    )
    t2 = b2.astype(np.float32) - m2.astype(np.float32) * s2
    bn2 = np.zeros((128, 2 * CG), dtype=np.float32)
    for cg in range(CG):
        bn2[:, cg * 2] = s2[cg * 128 : (cg + 1) * 128]
        bn2[:, cg * 2 + 1] = t2[cg * 128 : (cg + 1) * 128]

    return {"wdw": wdw, "wpw": wpw, "bn1": bn1, "bn2": bn2}


def kernel(x, w_dw, w_pw, g1, b1, m1, v1, g2, b2, m2, v2, _trace=False, _tmpdir=None):
    x = np.asarray(x, dtype=np.float32).astype(NP_BF16)
    shared = make_host_inputs(
        np.asarray(w_dw), np.asarray(w_pw),
        np.asarray(g1), np.asarray(b1), np.asarray(m1), np.asarray(v1),
        np.asarray(g2), np.asarray(b2), np.asarray(m2), np.asarray(v2),
    )
    in_maps = []
    for i in range(N_CORES):
        m = {"x": np.ascontiguousarray(x[i * BS : (i + 1) * BS])}
        m.update(shared)
        in_maps.append(m)

    nc = _get_nc()
    res = run_bass_kernel_spmd(
        nc, in_maps, core_ids=list(range(N_CORES)), trace=_trace, tmpdir=_tmpdir
    )
    y = np.concatenate(
        [res.results[i]["y"].astype(np.float32) for i in range(N_CORES)], axis=0
    )
    if _trace:
        return y, res
    return y
